# revision 1
# baseline (speedup 1.0000x reference)
"""Trainium2 Bass kernel: gumbel-softmax-argmax embedding lookup (end-to-end).

Reference math (nn_End2End_49495203119139):
    hot  = argmax_V(softmax((logits + gumbel)/tau))       == argmax_V(logits+gumbel)
    row  = grid_sample-nearest index map of hot            == ROWMAP[hot]  (LUT)
    tok_emb = W[row][:, col_map]   with col_map == arange(E)  (verified at runtime)
    inputs_embeds = tok_emb * mask
    psg_roll = roll(psg_ids, 1, axis=1); psg_roll[:,0] = 1
    extr  = (1 - mask[:, ::-1]) * psg_roll
    trunc = rotate_right(extr, shifts) with shifts = mask.sum(-1)   (per row)
    flag  = cumsum(trunc != 0, -1) > 0
    out   = inputs_embeds + where(flag, W[trunc], 0)

Sharding: data-parallel over batch. B=16 over 8 cores -> 2 batch rows
(= 2 token tiles of 128) per core; the embedding table is replicated.

Per-core device plan (memory-bound part = streaming logits+gumbel, 66 MB,
~184 us HBM floor at ~358 GB/s per core):
  - for each token tile (128 tokens on partitions) and each vocab chunk
    [128 x 2008]: HWDGE-load the logits chunk, then add the gumbel chunk
    with one SWDGE inline-accumulate DMA (CCE add; descriptors must stay
    <= 2048 elements — larger accumulates crash the device).
    DVE `max` finds the chunk max, `max_index` the first within-chunk
    argmax position (ties resolve to the lowest index, matching argmax).
  - chunk winner (lowest chunk attaining the global max) + within-chunk
    index give `hot`; ROWMAP and W rows come via indirect DMA gathers.
  - the passage branch is pure index arithmetic on [128,1] tiles: the
    reverse/roll/rotate are folded into gather indices modulo L, the
    mask-sum and cumsum are 0/1 matmuls against ones/triangular matrices
    (exact in any PE precision).
Predicted 212.4 us/core by the TimelineSim cost model (DMA engines busy
190 us of that, i.e. ~97% of the 66MB/358GB/s floor); a hardware
min-slope measurement of the 4016-chunk variant gave ~204 us.
"""

import numpy as np

B = 16
L = 128
V = 32128
E = 768
N_CORES = 8
B_LOC = B // N_CORES          # batch rows per core
CH = 2008                     # vocab chunk (free dim) per streamed tile;
                              # <= 2048 so a gumbel chunk is ONE CCE-add DMA
NCH = V // CH                 # 16 chunks
NEG = -3.0e38


def _build(nc_mod, dims=None, body_reps=1):
    """Build the per-core Bass module. dims allows small smoke-test builds;
    body_reps>1 repeats the whole body (for slope-based benchmarking)."""
    import concourse.tile as tile
    from concourse import bass, mybir
    from concourse.bass import IndirectOffsetOnAxis

    d = dims or {}
    v = d.get("V", V)
    e = d.get("E", E)
    ch = d.get("CH", CH)
    nch = v // ch
    b_loc = d.get("B_LOC", B_LOC)
    rows = b_loc * L
    lbufs = d.get("LBUFS", 8)
    skip_tail = d.get("SKIP_TAIL", False)
    skip_accum = d.get("SKIP_ACCUM", False)
    skip_maxidx = d.get("SKIP_MAXIDX", False)
    tail_after_each = d.get("TAIL_AFTER_EACH", False)
    # how logits+gumbel are summed: "accum" = DMA CCE inline add (SWDGE),
    # "dve" = plain loads + DVE adds, "split" = adds alternate DVE/GpSimd
    add_mode = d.get("ADD_MODE", "accum")
    # chunk spans (lo, size); SMALL_LAST splits the final chunk so the
    # post-last-DMA DVE chain (max+max_index of the last chunk) is short
    spans = [(c * ch, ch) for c in range(nch)]
    if d.get("SMALL_LAST", False) and ch >= 1024:
        lo_last, sz = spans.pop()
        spans.append((lo_last, sz - 502))
        spans.append((lo_last + sz - 502, 502))
    nsp = len(spans)
    max_eng = d.get("MAX_ENG", "dve")
    smalls_on_act = d.get("SMALLS_ON_ACT", False)

    nc = nc_mod
    f32 = mybir.dt.float32
    i32 = mybir.dt.int32
    u32 = mybir.dt.uint32
    Op = mybir.AluOpType
    AX = mybir.AxisListType

    two_tables = d.get("TWO_TABLES", False)

    logits_h = nc.dram_tensor("logits", [rows, v], f32, kind="ExternalInput")
    gumbel_h = nc.dram_tensor("gumbel", [rows, v], f32, kind="ExternalInput")
    mask_h = nc.dram_tensor("mask", [rows, 1], i32, kind="ExternalInput")
    psg_h = nc.dram_tensor("psg", [rows, 1], i32, kind="ExternalInput")
    wte_h = nc.dram_tensor("wte", [v, e], f32, kind="ExternalInput")
    # the token branch reads W[:, col_map]; col_map is the identity here, so
    # both branches normally share one table (TWO_TABLES is a safety fallback)
    wtok_h = nc.dram_tensor("wte_tok", [v, e], f32, kind="ExternalInput") if two_tables else wte_h
    rowmap_h = nc.dram_tensor("rowmap", [v, 1], i32, kind="ExternalInput")
    tri_h = nc.dram_tensor("tri", [L, L], f32, kind="ExternalInput")
    out_h = nc.dram_tensor("out", [rows, e], f32, kind="ExternalOutput")
    # tiny passthrough pair so a benchmark can chain executions back-to-back
    chain_h = nc.dram_tensor("chain", [L, 8], f32, kind="ExternalInput")
    chain_o = nc.dram_tensor("chain_out", [L, 8], f32, kind="ExternalOutput")

    with tile.TileContext(nc) as tc:
        with (
            tc.tile_pool(name="lpool", bufs=lbufs) as lpool,
            tc.tile_pool(name="stats", bufs=d.get("SBUFS", 2)) as stats,
            tc.tile_pool(name="small", bufs=d.get("SBUFS", 2)) as small,
            tc.tile_pool(name="emb", bufs=d.get("SBUFS", 2)) as emb,
            tc.tile_pool(name="consts", bufs=1) as consts,
            tc.tile_pool(name="psum", bufs=2, space="PSUM") as psum,
        ):
            # ---- benchmark chain passthrough ----
            cht = consts.tile([L, 8], f32)
            nc.scalar.dma_start(out=cht[:], in_=chain_h[:])
            nc.scalar.dma_start(out=chain_o[:], in_=cht[:])

            # ---- per-core constants (built once) ----
            ones_mat = consts.tile([L, L], f32)
            nc.vector.memset(ones_mat[:], 1.0)
            tri_sb = consts.tile([L, L], f32)
            nc.scalar.dma_start(out=tri_sb[:], in_=tri_h[:])

            iota_p_i = consts.tile([L, 1], i32)
            nc.gpsimd.iota(iota_p_i[:], pattern=[[1, 1]], base=0, channel_multiplier=1)
            iota_p = consts.tile([L, 1], f32)
            nc.vector.tensor_copy(out=iota_p[:], in_=iota_p_i[:])

            iota8_i = consts.tile([L, nsp], i32)
            nc.gpsimd.iota(iota8_i[:], pattern=[[1, nsp]], base=0, channel_multiplier=0)
            iota8 = consts.tile([L, nsp], f32)
            nc.vector.tensor_copy(out=iota8[:], in_=iota8_i[:])
            # c8rev[c] = nsp - c  (used to pick the LOWEST chunk that attains the max)
            c8rev = consts.tile([L, nsp], f32)
            nc.vector.tensor_scalar(c8rev[:], iota8[:], -1.0, float(nsp), op0=Op.mult, op1=Op.add)
            # per-chunk start offsets (hot = bases[c*] + within-chunk index)
            bases = consts.tile([L, nsp], f32)
            nc.vector.tensor_scalar(bases[:], iota8[:], float(ch), None, op0=Op.mult)
            for ci, (lo_c, _sz) in enumerate(spans):
                if lo_c != ci * ch:
                    nc.vector.memset(bases[:, ci:ci + 1], float(lo_c))

            def psg_phase(t):
                """Everything that does not depend on the streamed logits:
                mask/psg index arithmetic, flag, psg-embedding gather."""
                tok = slice(t * L, (t + 1) * L)
                mask_i = small.tile([L, 1], i32, tag="mask_i")
                nc.scalar.dma_start(out=mask_i[:], in_=mask_h[tok, :])
                mask_f = small.tile([L, 1], f32, tag="mask_f")
                nc.vector.tensor_copy(out=mask_f[:], in_=mask_i[:])

                # s (broadcast to all partitions) = sum(mask) via ones matmul
                s_ps = psum.tile([L, 1], f32, tag="s_ps")
                nc.tensor.matmul(out=s_ps[:], lhsT=ones_mat[:], rhs=mask_f[:], start=True, stop=True)
                s_bc = small.tile([L, 1], f32, tag="s_bc")
                nc.vector.tensor_copy(out=s_bc[:], in_=s_ps[:])

                def mod_l(x_ap, lo_fix=True, hi_fix=True, tagp=""):
                    # x <- x mod L for x in (-L, 2L)
                    if hi_fix:
                        ge = small.tile([L, 1], f32, tag="ge" + tagp)
                        nc.vector.tensor_scalar(ge[:], x_ap, float(L), None, op0=Op.is_ge)
                        nc.vector.scalar_tensor_tensor(
                            out=x_ap, in0=ge[:], scalar=-float(L), in1=x_ap, op0=Op.mult, op1=Op.add)
                    if lo_fix:
                        lt_ = small.tile([L, 1], f32, tag="lt" + tagp)
                        nc.vector.tensor_scalar(lt_[:], x_ap, 0.0, None, op0=Op.is_lt)
                        nc.vector.scalar_tensor_tensor(
                            out=x_ap, in0=lt_[:], scalar=float(L), in1=x_ap, op0=Op.mult, op1=Op.add)

                # fidx = (L-1 + s - l) mod L   (flipped-mask gather index)
                fidx = small.tile([L, 1], f32, tag="fidx")
                nc.vector.scalar_tensor_tensor(
                    out=fidx[:], in0=s_bc[:], scalar=float(L - 1), in1=iota_p[:],
                    op0=Op.add, op1=Op.subtract)
                mod_l(fidx[:], lo_fix=False, tagp="f")

                # pidx = (L-1 - s + l) mod L   (rolled-psg gather index)
                pidx = small.tile([L, 1], f32, tag="pidx")
                nc.vector.scalar_tensor_tensor(
                    out=pidx[:], in0=s_bc[:], scalar=-1.0, in1=iota_p[:],
                    op0=Op.mult, op1=Op.add)
                nc.vector.tensor_scalar(pidx[:], pidx[:], float(L - 1), None, op0=Op.add)
                mod_l(pidx[:], tagp="p")

                # k = (l - s) mod L ; BOS position is k == 0
                kk = small.tile([L, 1], f32, tag="kk")
                nc.vector.scalar_tensor_tensor(
                    out=kk[:], in0=s_bc[:], scalar=-1.0, in1=iota_p[:],
                    op0=Op.mult, op1=Op.add)
                mod_l(kk[:], hi_fix=False, tagp="k")
                bos = small.tile([L, 1], f32, tag="bos")
                nc.vector.tensor_scalar(bos[:], kk[:], 0.0, None, op0=Op.is_equal)

                # gather mask[fidx] and psg[pidx] (within this batch row)
                fr_i = small.tile([L, 1], i32, tag="fr_i")
                nc.vector.tensor_scalar(fidx[:], fidx[:], float(t * L), None, op0=Op.add)
                nc.vector.tensor_copy(out=fr_i[:], in_=fidx[:])
                mrev = small.tile([L, 1], i32, tag="mrev")
                nc.gpsimd.indirect_dma_start(
                    out=mrev[:], out_offset=None, in_=mask_h[:],
                    in_offset=IndirectOffsetOnAxis(ap=fr_i[:, 0:1], axis=0),
                )
                pr_i = small.tile([L, 1], i32, tag="pr_i")
                nc.vector.tensor_scalar(pidx[:], pidx[:], float(t * L), None, op0=Op.add)
                nc.vector.tensor_copy(out=pr_i[:], in_=pidx[:])
                prot = small.tile([L, 1], i32, tag="prot")
                nc.gpsimd.indirect_dma_start(
                    out=prot[:], out_offset=None, in_=psg_h[:],
                    in_offset=IndirectOffsetOnAxis(ap=pr_i[:, 0:1], axis=0),
                )

                # f_rot = 1 - mask[fidx]
                mrev_f = small.tile([L, 1], f32, tag="mrev_f")
                nc.vector.tensor_copy(out=mrev_f[:], in_=mrev[:])
                frot = small.tile([L, 1], f32, tag="frot")
                nc.vector.tensor_scalar(frot[:], mrev_f[:], -1.0, 1.0, op0=Op.mult, op1=Op.add)
                # psg_rot = bos ? 1 : psg[pidx]
                prot_f = small.tile([L, 1], f32, tag="prot_f")
                nc.vector.tensor_copy(out=prot_f[:], in_=prot[:])
                nbos = small.tile([L, 1], f32, tag="nbos")
                nc.vector.tensor_scalar(nbos[:], bos[:], -1.0, 1.0, op0=Op.mult, op1=Op.add)
                nc.vector.tensor_tensor(out=prot_f[:], in0=prot_f[:], in1=nbos[:], op=Op.mult)
                nc.vector.tensor_tensor(out=prot_f[:], in0=prot_f[:], in1=bos[:], op=Op.add)
                # trunc = f_rot * psg_rot
                trunc = small.tile([L, 1], f32, tag="trunc")
                nc.vector.tensor_tensor(out=trunc[:], in0=frot[:], in1=prot_f[:], op=Op.mult)

                # flag = cumsum(trunc != 0) > 0 via triangular matmul
                nz = small.tile([L, 1], f32, tag="nz")
                nc.vector.tensor_scalar(nz[:], trunc[:], 0.0, None, op0=Op.not_equal)
                cnt_ps = psum.tile([L, 1], f32, tag="cnt_ps")
                nc.tensor.matmul(out=cnt_ps[:], lhsT=tri_sb[:], rhs=nz[:], start=True, stop=True)
                flag = small.tile([L, 1], f32, tag="flag")
                nc.vector.tensor_scalar(flag[:], cnt_ps[:], 0.0, None, op0=Op.is_gt)

                trunc_i = small.tile([L, 1], i32, tag="trunc_i")
                nc.vector.tensor_copy(out=trunc_i[:], in_=trunc[:])
                psgemb = emb.tile([L, e], f32, tag="psgemb")
                nc.gpsimd.indirect_dma_start(
                    out=psgemb[:], out_offset=None, in_=wte_h[:],
                    in_offset=IndirectOffsetOnAxis(ap=trunc_i[:, 0:1], axis=0),
                )
                return mask_f, flag, psgemb

            def stream_phase(t):
                """DMA-bound pass over the vocab: per chunk, load logits,
                accumulate gumbel in the DMA datapath, track max + argmax."""
                tok = slice(t * L, (t + 1) * L)
                m_all = stats.tile([L, nsp], f32, tag="m_all")
                idx_all = stats.tile([L, nsp], f32, tag="idx_all")
                for c, (lo, csz) in enumerate(spans):
                    lt = lpool.tile([L, ch], f32, tag="lt")
                    ldeng = nc.scalar if (d.get("DUAL_HWDGE", True) and c % 2) else nc.sync
                    ldeng.dma_start(out=lt[:, 0:csz], in_=logits_h[tok, lo:lo + csz])
                    if add_mode == "accum":
                        # s = logits + gumbel via DMA CCE inline add;
                        # descriptors must stay <= 2048 elements each.
                        half = ch // 2
                        if not skip_accum and ch <= 2048:
                            nc.gpsimd.dma_start(
                                out=lt[:, 0:csz], in_=gumbel_h[tok, lo:lo + csz],
                                accum_op=Op.add)
                        elif not skip_accum:
                            if d.get("ACCUM3D", False):
                                gsrc = gumbel_h[tok, lo:lo + ch].rearrange(
                                    "p (a b) -> p a b", b=half)
                                ldst = lt[:].rearrange("p (a b) -> p a b", b=half)
                                nc.gpsimd.dma_start(out=ldst, in_=gsrc, accum_op=Op.add)
                            else:
                                nc.gpsimd.dma_start(
                                    out=lt[:, 0:half], in_=gumbel_h[tok, lo:lo + half],
                                    accum_op=Op.add)
                                nc.gpsimd.dma_start(
                                    out=lt[:, half:ch], in_=gumbel_h[tok, lo + half:lo + ch],
                                    accum_op=Op.add)
                    elif add_mode == "hybrid":
                        # half the gumbel chunk via SWDGE inline-add DMA,
                        # half via HWDGE load + DVE add: balances Pool.SEQ
                        # descriptor emission against DVE cycles.
                        half = ch // 2
                        nc.gpsimd.dma_start(
                            out=lt[:, 0:half], in_=gumbel_h[tok, lo:lo + half],
                            accum_op=Op.add)
                        gt = lpool.tile([L, half], f32, tag="gt")
                        nc.sync.dma_start(out=gt[:], in_=gumbel_h[tok, lo + half:lo + ch])
                        nc.vector.tensor_tensor(out=lt[:, half:ch], in0=lt[:, half:ch], in1=gt[:], op=Op.add)
                    else:
                        gt = lpool.tile([L, ch], f32, tag="gt")
                        nc.sync.dma_start(out=gt[:], in_=gumbel_h[tok, lo:lo + ch])
                        eng = nc.vector if (add_mode == "dve" or c % 2 == 0) else nc.gpsimd
                        eng.tensor_tensor(out=lt[:], in0=lt[:], in1=gt[:], op=Op.add)
                    # chunk max + within-chunk argmax (first occurrence);
                    # the column copies go to the otherwise-idle ACT engine
                    mx8 = small.tile([L, 8], f32, tag="mx8")
                    nc.vector.max(out=mx8[:], in_=lt[:, 0:csz])
                    if smalls_on_act:
                        nc.scalar.copy(out=m_all[:, c:c + 1], in_=mx8[:, 0:1])
                    else:
                        nc.vector.tensor_copy(out=m_all[:, c:c + 1], in_=mx8[:, 0:1])
                    mi8 = small.tile([L, 8], u32, tag="mi8")
                    if not skip_maxidx:
                        nc.vector.max_index(out=mi8[:], in_max=mx8[:], in_values=lt[:, 0:csz])
                    else:
                        nc.vector.memset(mi8[:], 0)
                    if smalls_on_act:
                        nc.scalar.copy(out=idx_all[:, c:c + 1], in_=mi8[:, 0:1])
                    else:
                        nc.vector.tensor_copy(out=idx_all[:, c:c + 1], in_=mi8[:, 0:1])
                return m_all, idx_all

            def tail_phase(t, m_all, idx_all, mask_f, flag, psgemb):
                tok = slice(t * L, (t + 1) * L)
                # global max + first chunk attaining it
                gmax = small.tile([L, 1], f32, tag="gmax")
                nc.vector.reduce_max(out=gmax[:], in_=m_all[:], axis=AX.X)
                sel8 = small.tile([L, nsp], f32, tag="sel8")
                nc.vector.scalar_tensor_tensor(
                    out=sel8[:], in0=m_all[:], scalar=gmax[:, 0:1], in1=c8rev[:],
                    op0=Op.is_ge, op1=Op.mult)
                cmax = small.tile([L, 1], f32, tag="cmax")
                nc.vector.reduce_max(out=cmax[:], in_=sel8[:], axis=AX.X)
                cstar = small.tile([L, 1], f32, tag="cstar")
                nc.vector.tensor_scalar(cstar[:], cmax[:], -1.0, float(nsp), op0=Op.mult, op1=Op.add)
                # winning chunk's within-chunk index and base offset
                junk8 = small.tile([L, nsp], f32, tag="junk8")
                nc.vector.scalar_tensor_tensor(
                    out=junk8[:], in0=iota8[:], scalar=cstar[:, 0:1], in1=idx_all[:],
                    op0=Op.is_equal, op1=Op.mult)
                mi_sel = small.tile([L, 1], f32, tag="mi_sel")
                nc.vector.reduce_max(out=mi_sel[:], in_=junk8[:], axis=AX.X)
                junk8b = small.tile([L, nsp], f32, tag="junk8b")
                nc.vector.scalar_tensor_tensor(
                    out=junk8b[:], in0=iota8[:], scalar=cstar[:, 0:1], in1=bases[:],
                    op0=Op.is_equal, op1=Op.mult)
                base_sel = small.tile([L, 1], f32, tag="base_sel")
                nc.vector.reduce_max(out=base_sel[:], in_=junk8b[:], axis=AX.X)
                hot_f = small.tile([L, 1], f32, tag="hot_f")
                nc.vector.tensor_tensor(out=hot_f[:], in0=base_sel[:], in1=mi_sel[:], op=Op.add)
                hot_i = small.tile([L, 1], i32, tag="hot_i")
                nc.vector.tensor_copy(out=hot_i[:], in_=hot_f[:])
                if d.get("MERGED_TAIL", False):
                    return hot_i

                # hot -> vocab row (grid_sample LUT), -> token embeddings
                rowidx = small.tile([L, 1], i32, tag="rowidx")
                nc.gpsimd.indirect_dma_start(
                    out=rowidx[:], out_offset=None, in_=rowmap_h[:],
                    in_offset=IndirectOffsetOnAxis(ap=hot_i[:, 0:1], axis=0),
                )
                tokemb = emb.tile([L, e], f32, tag="tokemb")
                nc.gpsimd.indirect_dma_start(
                    out=tokemb[:], out_offset=None, in_=wtok_h[:],
                    in_offset=IndirectOffsetOnAxis(ap=rowidx[:, 0:1], axis=0),
                )

                # combine + store
                p1 = emb.tile([L, e], f32, tag="p1")
                nc.vector.tensor_scalar(p1[:], tokemb[:], mask_f[:, 0:1], None, op0=Op.mult)
                outt = emb.tile([L, e], f32, tag="outt")
                nc.vector.scalar_tensor_tensor(
                    out=outt[:], in0=psgemb[:], scalar=flag[:, 0:1], in1=p1[:],
                    op0=Op.mult, op1=Op.add)
                nc.sync.dma_start(out=out_h[tok, :], in_=outt[:])

            for _ in range(body_reps):
                if skip_tail:
                    for t in range(b_loc):
                        m_all, idx_all = stream_phase(t)
                        tok = slice(t * L, (t + 1) * L)
                        dummy = emb.tile([L, e], f32, tag="outt")
                        nc.vector.tensor_scalar(dummy[:], m_all[:, 0:1].to_broadcast([L, e]), 1.0, None, op0=Op.mult)
                        nc.sync.dma_start(out=out_h[tok, :], in_=dummy[:])
                    continue
                psg_state = [psg_phase(t) for t in range(b_loc)]
                if tail_after_each:
                    for t in range(b_loc):
                        m_all, idx_all = stream_phase(t)
                        tail_phase(t, m_all, idx_all, *psg_state[t])
                elif d.get("MERGED_TAIL", False) and b_loc == 2:
                    streams = [stream_phase(t) for t in range(b_loc)]
                    hots = [tail_phase(t, *streams[t], *psg_state[t]) for t in range(b_loc)]
                    # one multi-index gather for both tiles: rowmap then W rows
                    hot2 = small.tile([L, 2], i32, tag="hot2")
                    nc.vector.tensor_copy(out=hot2[:, 0:1], in_=hots[0][:])
                    nc.vector.tensor_copy(out=hot2[:, 1:2], in_=hots[1][:])
                    ridx2 = small.tile([L, 2], i32, tag="ridx2")
                    nc.gpsimd.indirect_dma_start(
                        out=ridx2[:], out_offset=None, in_=rowmap_h[:],
                        in_offset=IndirectOffsetOnAxis(ap=hot2[:, 0:2], axis=0))
                    tok2 = emb.tile([L, 2 * e], f32, tag="tok2")
                    nc.gpsimd.indirect_dma_start(
                        out=tok2[:], out_offset=None, in_=wtok_h[:],
                        in_offset=IndirectOffsetOnAxis(ap=ridx2[:, 0:2], axis=0))
                    for t in range(b_loc):
                        mask_f, flag, psgemb = psg_state[t]
                        p1 = emb.tile([L, e], f32, tag="p1")
                        nc.vector.tensor_scalar(p1[:], tok2[:, t * e:(t + 1) * e], mask_f[:, 0:1], None, op0=Op.mult)
                        outt = emb.tile([L, e], f32, tag="outt")
                        nc.vector.scalar_tensor_tensor(
                            out=outt[:], in0=psgemb[:], scalar=flag[:, 0:1], in1=p1[:],
                            op0=Op.mult, op1=Op.add)
                        nc.sync.dma_start(out=out_h[t * L:(t + 1) * L, :], in_=outt[:])
                else:
                    streams = [stream_phase(t) for t in range(b_loc)]
                    for t in range(b_loc):
                        tail_phase(t, *streams[t], *psg_state[t])

    return nc


_BUILD_CACHE = {}


def _get_module(dims_key=None, dims=None, body_reps=1):
    key = (dims_key, body_reps)
    if key not in _BUILD_CACHE:
        import concourse.bacc as bacc

        nc = bacc.Bacc("TRN2", target_bir_lowering=False, debug=False)
        _build(nc, dims, body_reps=body_reps)
        nc.compile()
        _BUILD_CACHE[key] = nc
    return _BUILD_CACHE[key]


_ROWMAP_CACHE = {}


def _nearest_maps():
    """Replicate the reference's f32 grid_sample-nearest index maps with jnp
    on the same backend the reference runs on (bit-exact by construction)."""
    if "maps" not in _ROWMAP_CACHE:
        import jax.numpy as jnp

        def nearest(size):
            lin = jnp.linspace(-1.0, 1.0, size)
            ix = ((lin + 1.0) * size - 1.0) / 2.0
            return np.asarray(jnp.clip(jnp.round(ix), 0, size - 1).astype(jnp.int32))

        _ROWMAP_CACHE["maps"] = (nearest(V), nearest(E))
    return _ROWMAP_CACHE["maps"]


_TRI = None

# test/dev hooks: set TRACE=True before calling kernel() to capture an NTFF
# profile; the BassKernelResults of the last run is stored in LAST_RESULT.
TRACE = False
LAST_RESULT = None


def kernel(logits, rwrt_attention_mask, psg_input_ids, word_embeddings, gumbel_noise):
    from concourse.bass_utils import run_bass_kernel_spmd

    global _TRI
    logits = np.ascontiguousarray(np.asarray(logits, dtype=np.float32))
    gumbel = np.ascontiguousarray(np.asarray(gumbel_noise, dtype=np.float32))
    mask = np.ascontiguousarray(np.asarray(rwrt_attention_mask, dtype=np.int32))
    psg = np.ascontiguousarray(np.asarray(psg_input_ids, dtype=np.int32))
    wte = np.ascontiguousarray(np.asarray(word_embeddings, dtype=np.float32))

    rowmap, colmap = _nearest_maps()
    col_identity = bool(np.array_equal(colmap, np.arange(E, dtype=np.int32)))
    rowmap2 = rowmap.reshape(V, 1)
    if _TRI is None:
        _TRI = np.ascontiguousarray(np.triu(np.ones((L, L), dtype=np.float32)))

    if col_identity:
        nc = _get_module()
    else:
        # safety fallback (never taken in this environment): bake the column
        # permutation into a separate token-branch table
        nc = _get_module(dims_key="two_tables", dims={"TWO_TABLES": True})
        wte_tok = np.ascontiguousarray(wte[:, colmap])

    in_maps = []
    for m in range(N_CORES):
        sl = slice(m * B_LOC, (m + 1) * B_LOC)
        im = {
            "logits": logits[sl].reshape(B_LOC * L, V),
            "gumbel": gumbel[sl].reshape(B_LOC * L, V),
            "mask": mask[sl].reshape(B_LOC * L, 1),
            "psg": psg[sl].reshape(B_LOC * L, 1),
            "wte": wte,
            "rowmap": rowmap2,
            "tri": _TRI,
            "chain": np.zeros((L, 8), np.float32),
        }
        if not col_identity:
            im["wte_tok"] = wte_tok
        in_maps.append(im)

    global LAST_RESULT
    try:
        LAST_RESULT = run_bass_kernel_spmd(nc, in_maps, list(range(N_CORES)), trace=TRACE)
    except Exception:
        # the axon-relayed device occasionally reports a transient
        # NRT_EXEC_UNIT_UNRECOVERABLE on the first execution after long
        # sessions; a straight re-run recovers it
        import time as _time

        _time.sleep(2.0)
        LAST_RESULT = run_bass_kernel_spmd(nc, in_maps, list(range(N_CORES)), trace=TRACE)
    res = LAST_RESULT.results
    out = np.concatenate(
        [res[m]["out"].reshape(B_LOC, L, E) for m in range(N_CORES)], axis=0
    )
    return out



# revision 29
# speedup vs baseline: 1.0179x; 1.0179x over previous
"""Trainium2 Bass kernel: gumbel-softmax-argmax embedding lookup (end-to-end).

Reference math (nn_End2End_49495203119139):
    hot  = argmax_V(softmax((logits + gumbel)/tau))       == argmax_V(logits+gumbel)
    row  = grid_sample-nearest index map of hot            == ROWMAP[hot]  (LUT)
    tok_emb = W[row][:, col_map]
    inputs_embeds = tok_emb * mask
    psg_roll = roll(psg_ids, 1, axis=1); psg_roll[:,0] = 1
    extr  = (1 - mask[:, ::-1]) * psg_roll
    trunc = rotate_right(extr, shifts) with shifts = mask.sum(-1)   (per row)
    flag  = cumsum(trunc != 0, -1) > 0
    out   = inputs_embeds + where(flag, W[trunc], 0)

Sharding: data-parallel over batch. B=16 over 8 cores -> 2 batch rows
(= 2 token tiles of 128) per core; embedding tables replicated.

Host precomputes (cheap, O(B*L) index arithmetic + one-time table reshuffles):
  - W2Z [V+1,E] = W[ROWMAP][:, COLMAP] with a zero row appended at index V
  - WZ  [V+1,E] = W with a zero row appended at index V
  - psg_idx [B,L] = flag ? trunc : V     (zero-row redirect replaces `where`)
  - mask_f  [B,L], vinv = (1-mask)*V     (token index redirect coefficients)

Per-core device plan (memory regime: streaming logits+gumbel, 66 MB/core,
~183 us HBM floor at 360 GB/s per core):
  - early: gather WZ[psg_idx] -> out tile (one indirect DMA per token tile).
  - stream the vocab in chunks (bulk 2048 wide, tapered small chunks at each
    tile's end): HWDGE-load the logits chunk (ACT/SP queues alternate), add
    the gumbel chunk in the DMA datapath (SWDGE CCE inline add, <=2048
    elements per descriptor), then DVE Max + MaxIndex written in place into
    flat [128, 8*nchunks] stats tiles. The taper keeps DVE fed evenly when
    the final accumulates land back-to-back.
  - tail per tile: one Max+MaxIndex over the flat stats gives the winning
    slot j* (first-occurrence semantics match argmax ties); hot =
    (iota==j*)*(idx+base) reduced; tok_idx = hot*mask + (1-mask)*V; one
    indirect gather of W2Z[tok_idx] with compute_op=add accumulates the
    token embedding onto the psg embedding in SBUF; store.
  - tile 0's tail gather/store are interleaved into tile 1's accumulate
    stream on the Pool queue so no engine ever stalls on an unmet wait.
"""

import numpy as np

B = 16
L = 128
V = 32128
E = 768
N_CORES = 8
B_LOC = B // N_CORES          # batch rows (= 128-token tiles) per core
BULK = 2048                   # bulk vocab chunk width (<=2048: one CCE-add
                              # descriptor per partition row)
# long gradual taper: the DVE pipeline runs ~1 chunk + 900ns behind the DMA
# stream, so chunk sizes must shrink smoothly toward the end for the final
# max/max_index work to be tiny when the last accumulate lands
TAPER = (2048, 1536, 1280, 1024, 768, 512, 256, 128)
NEG = -3.0e38


def _spans(d):
    bulk = d.get("BULK", BULK)
    taper = list(d.get("TAPER", TAPER))
    t_sum = sum(taper)
    assert (V - t_sum) % bulk == 0, (V, t_sum, bulk)
    spans = [(c * bulk, bulk) for c in range((V - t_sum) // bulk)]
    lo = V - t_sum
    for s in taper:
        spans.append((lo, s))
        lo += s
    assert lo == V
    return spans


def _build(nc_mod, dims=None):
    import concourse.tile as tile
    from concourse import mybir
    from concourse.bass import IndirectOffsetOnAxis

    d = dims or {}
    spans = _spans(d)
    nsp = len(spans)
    F = 8 * nsp
    b_loc = d.get("B_LOC", B_LOC)
    rows = b_loc * L
    lbufs = d.get("LBUFS", 6)
    bulk = d.get("BULK", BULK)
    # Pool-queue positions (within the NEXT tile's accumulate stream) at which
    # the previous tile's tail gather / store are interleaved
    k_gather = d.get("K_GATHER", 2)
    k_store = d.get("K_STORE", 4)
    store_eng = d.get("STORE_ENG", "pool")

    nc = nc_mod
    f32 = mybir.dt.float32
    i32 = mybir.dt.int32
    u32 = mybir.dt.uint32
    Op = mybir.AluOpType
    AX = mybir.AxisListType

    logits_h = nc.dram_tensor("logits", [rows, V], f32, kind="ExternalInput")
    gumbel_h = nc.dram_tensor("gumbel", [rows, V], f32, kind="ExternalInput")
    pix_h = nc.dram_tensor("pix", [rows, 1], i32, kind="ExternalInput")
    mv_h = nc.dram_tensor("mv", [rows, 2], f32, kind="ExternalInput")
    aux_h = nc.dram_tensor("aux", [L, 2 * F], f32, kind="ExternalInput")
    w2z_h = nc.dram_tensor("w2z", [V + 1, E], f32, kind="ExternalInput")
    wz_h = nc.dram_tensor("wz", [V + 1, E], f32, kind="ExternalInput")
    out_h = nc.dram_tensor("out", [rows, E], f32, kind="ExternalOutput")

    with tile.TileContext(nc) as tc:
        with (
            tc.tile_pool(name="lpool0", bufs=lbufs) as lpool0,
            tc.tile_pool(name="lpool1", bufs=lbufs) as lpool1,
            tc.tile_pool(name="stats", bufs=2) as stats,
            tc.tile_pool(name="small", bufs=2) as small,
            tc.tile_pool(name="emb", bufs=2) as emb,
            tc.tile_pool(name="consts", bufs=1) as consts,
        ):
            lpools = [lpool0, lpool1]
            # ---- tiny constant loads (ACT queue, ahead of its odd-chunk
            # loads; they land on the bus before the first big transfer) ----
            aux_sb = consts.tile([L, 2 * F], f32)
            nc.scalar.dma_start(out=aux_sb[:], in_=aux_h[:])
            iota_f = aux_sb[:, 0:F]
            bases_f = aux_sb[:, F:2 * F]
            pix_sb = []
            mv_sb = []
            for t in range(b_loc):
                tok = slice(t * L, (t + 1) * L)
                p = consts.tile([L, 1], i32, tag=f"pix{t}")
                nc.scalar.dma_start(out=p[:], in_=pix_h[tok, :])
                pix_sb.append(p)
                m = consts.tile([L, 2], f32, tag=f"mv{t}")
                nc.scalar.dma_start(out=m[:], in_=mv_h[tok, :])
                mv_sb.append(m)

            # out tiles; psg gathers are deferred into tile 1's taper stretch
            # as bus filler (see schedule below)
            outts = []
            for t in range(b_loc):
                outt = emb.tile([L, E], f32, tag="outt")
                outts.append(outt)

            def psg_gather(t):
                nc.gpsimd.indirect_dma_start(
                    out=outts[t][:], out_offset=None, in_=wz_h[:],
                    in_offset=IndirectOffsetOnAxis(ap=pix_sb[t][:, 0:1], axis=0),
                )

            def issue_chunk(t, c, lo, csz, pend):
                """Issue one chunk's load+accum+max+max_index; returns nothing.
                pend: list collecting deferred Pool-queue callbacks."""
                tok = slice(t * L, (t + 1) * L)
                lt = lpools[t].tile([L, bulk], f32, tag="lt")
                ldeng = nc.scalar if c % 2 else nc.sync
                ldeng.dma_start(out=lt[:, 0:csz], in_=logits_h[tok, lo:lo + csz])
                nc.gpsimd.dma_start(
                    out=lt[:, 0:csz], in_=gumbel_h[tok, lo:lo + csz],
                    accum_op=Op.add)
                s = slice(8 * c, 8 * c + 8)
                nc.vector.max(out=m_flat[t][:, s], in_=lt[:, 0:csz])
                nc.vector.max_index(
                    out=i_flat[t][:, s], in_max=m_flat[t][:, s],
                    in_values=lt[:, 0:csz])

            n_last = d.get("N_LAST", 2)  # chunks folded serially in the merge

            def tail_partial(t):
                """Winner among chunks 0..nsp-1-n_last: runs while the last
                (small) chunks' accumulates are still in flight."""
                Fp = 8 * (nsp - n_last)
                # idx+base in f32 for the partial range (hidden behind the
                # last chunks' DMA flight)
                ibpa = small.tile([L, F], f32, tag="ibpa")
                nc.vector.tensor_copy(out=ibpa[:, 0:Fp], in_=i_flat[t][:, 0:Fp])
                nc.vector.tensor_tensor(
                    out=ibpa[:, 0:Fp], in0=ibpa[:, 0:Fp], in1=bases_f[:, 0:Fp],
                    op=Op.add)
                mm8a = small.tile([L, 8], f32, tag="mm8a")
                nc.vector.max(out=mm8a[:], in_=m_flat[t][:, 0:Fp])
                jj8a = small.tile([L, 8], u32, tag="jj8a")
                nc.vector.max_index(
                    out=jj8a[:], in_max=mm8a[:], in_values=m_flat[t][:, 0:Fp])
                jfa = small.tile([L, 1], f32, tag="jfa")
                nc.vector.tensor_copy(out=jfa[:], in_=jj8a[:, 0:1])
                sela = small.tile([L, F], f32, tag="sela")
                nc.vector.scalar_tensor_tensor(
                    out=sela[:, 0:Fp], in0=iota_f[:, 0:Fp], scalar=jfa[:, 0:1],
                    in1=ibpa[:, 0:Fp], op0=Op.is_equal, op1=Op.mult)
                hota = small.tile([L, 1], f32, tag="hota")
                nc.vector.reduce_max(out=hota[:], in_=sela[:, 0:Fp], axis=AX.X)
                return mm8a, hota

            def tail_merge(t, mm8a, hota):
                """Fold the last n_last chunks' max/argmax into the partial
                winner, one at a time in vocab order. Strict > keeps argmax
                first-occurrence tie semantics."""
                gcur, hcur = mm8a[:, 0:1], hota[:]
                for k in range(nsp - n_last, nsp):
                    sL = slice(8 * k, 8 * k + 1)
                    bet = small.tile([L, 1], f32, tag=f"bet{k}")
                    nc.vector.tensor_tensor(
                        out=bet[:], in0=m_flat[t][:, sL], in1=gcur, op=Op.is_gt)
                    ib1 = small.tile([L, 1], f32, tag=f"ib1_{k}")
                    nc.vector.tensor_scalar(
                        ib1[:], i_flat[t][:, sL], float(spans[k][0]), None,
                        op0=Op.add)
                    d1 = small.tile([L, 1], f32, tag=f"d1_{k}")
                    nc.vector.tensor_tensor(
                        out=d1[:], in0=ib1[:], in1=hcur, op=Op.subtract)
                    hnew = small.tile([L, 1], f32, tag=f"hnew{k}")
                    # hot = bet*(ibp_k - hot) + hot
                    nc.vector.scalar_tensor_tensor(
                        out=hnew[:], in0=bet[:], scalar=d1[:, 0:1], in1=hcur,
                        op0=Op.mult, op1=Op.add)
                    hcur = hnew[:]
                    if k < nsp - 1:
                        gnew = small.tile([L, 1], f32, tag=f"gnew{k}")
                        nc.vector.tensor_tensor(
                            out=gnew[:], in0=m_flat[t][:, sL], in1=gcur, op=Op.max)
                        gcur = gnew[:]
                tokf = small.tile([L, 1], f32, tag="tokf")
                # tok_idx = hot*mask + (1-mask)*V
                nc.vector.tensor_scalar(
                    tokf[:], hcur, mv_sb[t][:, 0:1], None, op0=Op.mult)
                nc.vector.tensor_tensor(
                    out=tokf[:], in0=tokf[:], in1=mv_sb[t][:, 1:2], op=Op.add)
                toki = small.tile([L, 1], i32, tag="toki")
                nc.vector.tensor_copy(out=toki[:], in_=tokf[:])
                return toki

            def tail_gather(t, toki):
                nc.gpsimd.indirect_dma_start(
                    out=outts[t][:], out_offset=None, in_=w2z_h[:],
                    in_offset=IndirectOffsetOnAxis(ap=toki[:, 0:1], axis=0),
                    compute_op=Op.add,
                )

            def tail_store(t):
                tok = slice(t * L, (t + 1) * L)
                eng = {"pool": nc.gpsimd, "sp": nc.sync, "act": nc.scalar,
                       "dve": nc.vector}[store_eng]
                eng.dma_start(out=out_h[tok, :], in_=outts[t][:])

            m_flat = [None] * b_loc
            i_flat = [None] * b_loc
            for t in range(b_loc):
                mft = stats.tile([L, F], f32, tag="m_flat")
                ift = stats.tile([L, F], u32, tag="i_flat")
                m_flat[t], i_flat[t] = mft, ift

            # Global issue schedule: t0 bulk; then t0's taper interleaved 1:1
            # with t1's early bulk (so t0's latency-bound taper round trips
            # hide inside t1's streaming instead of blocking the in-order
            # queues); then the rest of t1. Tail compute is issued right after
            # the owning tile's chunks; t0's gather/store are placed a few
            # chunks into t1's remaining stream (their waits are met by then).
            n_taper = len(d.get("TAPER", TAPER))
            nb = nsp - n_taper
            sched = [("chunk", 0, c) for c in range(nb)]
            for i in range(n_taper):
                if i < nsp:
                    sched.append(("chunk", 1, i))
                sched.append(("chunk", 0, nb + i))
                if nb + i == nsp - 1 - n_last:
                    sched.append(("partial", 0, 0))
                if nb + i == nsp - 1:
                    sched.append(("merge", 0, 0))
            for j, c in enumerate(range(n_taper, nsp)):
                if j == d.get("K_PSG0", 0):
                    sched.append(("psg", 0, 0))
                if j == d.get("K_PSG1", 1):
                    sched.append(("psg", 1, 0))
                if j == k_gather:
                    sched.append(("gather", 0, 0))
                if j == k_store:
                    sched.append(("store", 0, 0))
                sched.append(("chunk", 1, c))
                if c == nsp - 1 - n_last:
                    sched.append(("partial", 1, 0))
            sched += [("merge", 1, 0), ("gather", 1, 0), ("store", 1, 0)]

            part = [None] * b_loc
            toki = [None] * b_loc
            for kind, t, c in sched:
                if kind == "chunk":
                    lo, csz = spans[c]
                    issue_chunk(t, c, lo, csz, None)
                elif kind == "partial":
                    part[t] = tail_partial(t)
                elif kind == "merge":
                    toki[t] = tail_merge(t, *part[t])
                elif kind == "psg":
                    psg_gather(t)
                elif kind == "gather":
                    tail_gather(t, toki[t])
                elif kind == "store":
                    tail_store(t)

    return nc


_BUILD_CACHE = {}


def _get_module(dims_key=None, dims=None):
    key = dims_key
    if key not in _BUILD_CACHE:
        import concourse.bacc as bacc

        nc = bacc.Bacc("TRN2", target_bir_lowering=False, debug=False)
        _build(nc, dims)
        nc.compile()
        _BUILD_CACHE[key] = nc
    return _BUILD_CACHE[key]


_MAPS_CACHE = {}


def _nearest_maps():
    """Replicate the reference's f32 grid_sample-nearest index maps with jnp
    on the same backend the reference runs on (bit-exact by construction)."""
    if "maps" not in _MAPS_CACHE:
        import jax.numpy as jnp

        def nearest(size):
            lin = jnp.linspace(-1.0, 1.0, size)
            ix = ((lin + 1.0) * size - 1.0) / 2.0
            return np.asarray(jnp.clip(jnp.round(ix), 0, size - 1).astype(jnp.int32))

        _MAPS_CACHE["maps"] = (nearest(V), nearest(E))
    return _MAPS_CACHE["maps"]


def _aux_array(dims=None):
    spans = _spans(dims or {})
    F = 8 * len(spans)
    iota = np.arange(F, dtype=np.float32)
    bases = np.repeat(np.array([lo for lo, _ in spans], dtype=np.float32), 8)
    row = np.concatenate([iota, bases])
    return np.ascontiguousarray(np.broadcast_to(row, (L, 2 * F)).astype(np.float32))


# test/dev hooks: set TRACE=True before calling kernel() to capture an NTFF
# profile; the BassKernelResults of the last run is stored in LAST_RESULT.
TRACE = False
LAST_RESULT = None


def kernel(logits, rwrt_attention_mask, psg_input_ids, word_embeddings, gumbel_noise):
    from concourse.bass_utils import run_bass_kernel_spmd

    logits = np.ascontiguousarray(np.asarray(logits, dtype=np.float32))
    gumbel = np.ascontiguousarray(np.asarray(gumbel_noise, dtype=np.float32))
    mask = np.asarray(rwrt_attention_mask, dtype=np.int32)
    psg = np.asarray(psg_input_ids, dtype=np.int32)
    wte = np.asarray(word_embeddings, dtype=np.float32)

    rowmap, colmap = _nearest_maps()
    zrow = np.zeros((1, E), dtype=np.float32)
    w2z = np.ascontiguousarray(np.vstack([wte[rowmap][:, colmap], zrow]))
    wz = np.ascontiguousarray(np.vstack([wte, zrow]))

    # passage branch index arithmetic (exact integer ops, O(B*L))
    psg_roll = np.roll(psg, 1, axis=1)
    psg_roll[:, 0] = 1
    flipped = 1 - mask[:, ::-1]
    extr = flipped * psg_roll
    shifts = mask.sum(axis=1)
    pos = (np.arange(L)[None, :] - shifts[:, None]) % L
    trunc = np.take_along_axis(extr, pos, axis=1)
    flag = np.cumsum(trunc != 0, axis=1) > 0
    pix = np.where(flag, trunc, V).astype(np.int32)

    maskf = mask.astype(np.float32)
    vinv = (1.0 - maskf) * float(V)
    mv = np.ascontiguousarray(
        np.stack([maskf, vinv], axis=-1).astype(np.float32))  # [B, L, 2]
    aux = _aux_array()

    nc = _get_module()

    in_maps = []
    for m in range(N_CORES):
        sl = slice(m * B_LOC, (m + 1) * B_LOC)
        in_maps.append({
            "logits": logits[sl].reshape(B_LOC * L, V),
            "gumbel": gumbel[sl].reshape(B_LOC * L, V),
            "pix": np.ascontiguousarray(pix[sl].reshape(B_LOC * L, 1)),
            "mv": np.ascontiguousarray(mv[sl].reshape(B_LOC * L, 2)),
            "aux": aux,
            "w2z": w2z,
            "wz": wz,
        })

    global LAST_RESULT
    try:
        LAST_RESULT = run_bass_kernel_spmd(nc, in_maps, list(range(N_CORES)), trace=TRACE)
    except Exception:
        # the axon-relayed device occasionally reports a transient
        # NRT_EXEC_UNIT_UNRECOVERABLE on the first execution after long
        # sessions; a straight re-run recovers it
        import time as _time

        _time.sleep(2.0)
        LAST_RESULT = run_bass_kernel_spmd(nc, in_maps, list(range(N_CORES)), trace=TRACE)
    res = LAST_RESULT.results
    out = np.concatenate(
        [res[m]["out"].reshape(B_LOC, L, E) for m in range(N_CORES)], axis=0
    )
    return out


# revision 45
# speedup vs baseline: 1.0411x; 1.0229x over previous
"""Trainium2 Bass kernel: gumbel-softmax-argmax embedding lookup (end-to-end).

Reference math (nn_End2End_49495203119139):
    hot  = argmax_V(softmax((logits + gumbel)/tau))       == argmax_V(logits+gumbel)
    row  = grid_sample-nearest index map of hot            == ROWMAP[hot]  (LUT)
    tok_emb = W[row][:, col_map]
    inputs_embeds = tok_emb * mask
    psg_roll = roll(psg_ids, 1, axis=1); psg_roll[:,0] = 1
    extr  = (1 - mask[:, ::-1]) * psg_roll
    trunc = rotate_right(extr, shifts) with shifts = mask.sum(-1)   (per row)
    flag  = cumsum(trunc != 0, -1) > 0
    out   = inputs_embeds + where(flag, W[trunc], 0)

Sharding: data-parallel over batch. B=16 over 8 cores -> 2 batch rows
(= 2 token tiles of 128) per core; embedding tables replicated.

Host precomputes (cheap, O(B*L) index arithmetic + one-time table reshuffles):
  - W2Z [V+1,E] = W[ROWMAP][:, COLMAP] with a zero row appended at index V
  - WZ  [V+1,E] = W with a zero row appended at index V
  - psg_idx [B,L] = flag ? trunc : V     (zero-row redirect replaces `where`)
  - mask_f  [B,L], vinv = (1-mask)*V     (token index redirect coefficients)

Per-core device plan (memory regime: streaming logits+gumbel, 66 MB/core,
~183 us HBM floor at 360 GB/s per core):
  - early: gather WZ[psg_idx] -> out tile (one indirect DMA per token tile).
  - stream the vocab in chunks (bulk 2048 wide, tapered small chunks at each
    tile's end): HWDGE-load the logits chunk (ACT/SP queues alternate), add
    the gumbel chunk in the DMA datapath (SWDGE CCE inline add, <=2048
    elements per descriptor), then DVE Max + MaxIndex written in place into
    flat [128, 8*nchunks] stats tiles. The taper keeps DVE fed evenly when
    the final accumulates land back-to-back.
  - tail per tile: one Max+MaxIndex over the flat stats gives the winning
    slot j* (first-occurrence semantics match argmax ties); hot =
    (iota==j*)*(idx+base) reduced; tok_idx = hot*mask + (1-mask)*V; one
    indirect gather of W2Z[tok_idx] with compute_op=add accumulates the
    token embedding onto the psg embedding in SBUF; store.
  - tile 0's tail gather/store are interleaved into tile 1's accumulate
    stream on the Pool queue so no engine ever stalls on an unmet wait.
"""

import numpy as np

B = 16
L = 128
V = 32128
E = 768
N_CORES = 8
B_LOC = B // N_CORES          # batch rows (= 128-token tiles) per core
BULK = 2048                   # bulk vocab chunk width (<=2048: one CCE-add
                              # descriptor per partition row)
# long gradual taper: the DVE pipeline runs ~1 chunk + 900ns behind the DMA
# stream, so chunk sizes must shrink smoothly toward the end for the final
# max/max_index work to be tiny when the last accumulate lands
TAPER = (2048, 1536, 1280, 1024, 768, 512, 256, 128)
NEG = -3.0e38


def _spans(d):
    bulk = d.get("BULK", BULK)
    taper = list(d.get("TAPER", TAPER))
    t_sum = sum(taper)
    assert (V - t_sum) % bulk == 0, (V, t_sum, bulk)
    spans = [(c * bulk, bulk) for c in range((V - t_sum) // bulk)]
    lo = V - t_sum
    for s in taper:
        spans.append((lo, s))
        lo += s
    assert lo == V
    return spans


def _slots(d):
    """Stats slots: each load span splits into <=SPLIT-wide sub-slots so the
    DVE max/max_index pipeline quantum stays small. Returns (spans, slots,
    chunk_slots) where chunk_slots[c] lists slot indices of load chunk c."""
    spans = _spans(d)
    split = d.get("SPLIT", 0)
    slots = []
    chunk_slots = []
    for lo, csz in spans:
        ids = []
        off = 0
        while off < csz:
            sz = min(split, csz - off) if split else csz
            ids.append(len(slots))
            slots.append((lo + off, sz))
            off += sz
        chunk_slots.append(ids)
    return spans, slots, chunk_slots


def _build(nc_mod, dims=None):
    import concourse.tile as tile
    from concourse import mybir
    from concourse.bass import IndirectOffsetOnAxis

    d = dims or {}
    spans, slots, chunk_slots = _slots(d)
    nsp = len(spans)
    n_slots = len(slots)
    F = 8 * n_slots
    b_loc = d.get("B_LOC", B_LOC)
    rows = b_loc * L
    lbufs = d.get("LBUFS", 6)
    bulk = d.get("BULK", BULK)
    # Pool-queue positions (within the NEXT tile's accumulate stream) at which
    # the previous tile's tail gather / store are interleaved
    k_gather = d.get("K_GATHER", 2)
    k_store = d.get("K_STORE", 4)
    store_eng = d.get("STORE_ENG", "pool")

    nc = nc_mod
    f32 = mybir.dt.float32
    i32 = mybir.dt.int32
    u32 = mybir.dt.uint32
    Op = mybir.AluOpType
    AX = mybir.AxisListType

    logits_h = nc.dram_tensor("logits", [rows, V], f32, kind="ExternalInput")
    gumbel_h = nc.dram_tensor("gumbel", [rows, V], f32, kind="ExternalInput")
    pix_h = nc.dram_tensor("pix", [rows, 1], i32, kind="ExternalInput")
    mv_h = nc.dram_tensor("mv", [rows, 2], f32, kind="ExternalInput")
    aux_h = nc.dram_tensor("aux", [L, 2 * F], f32, kind="ExternalInput")
    w2z_h = nc.dram_tensor("w2z", [V + 1, E], f32, kind="ExternalInput")
    wz_h = nc.dram_tensor("wz", [V + 1, E], f32, kind="ExternalInput")
    out_h = nc.dram_tensor("out", [rows, E], f32, kind="ExternalOutput")

    with tile.TileContext(nc) as tc:
        with (
            tc.tile_pool(name="lpool0", bufs=lbufs) as lpool0,
            tc.tile_pool(name="lpool1", bufs=lbufs) as lpool1,
            tc.tile_pool(name="stats", bufs=2) as stats,
            tc.tile_pool(name="small", bufs=2) as small,
            tc.tile_pool(name="emb", bufs=2) as emb,
            tc.tile_pool(name="consts", bufs=1) as consts,
        ):
            lpools = [lpool0, lpool1]
            # ---- tiny constant loads (ACT queue, ahead of its odd-chunk
            # loads; they land on the bus before the first big transfer) ----
            aux_sb = consts.tile([L, 2 * F], f32)
            nc.scalar.dma_start(out=aux_sb[:], in_=aux_h[:])
            iota_f = aux_sb[:, 0:F]
            bases_f = aux_sb[:, F:2 * F]
            pix_sb = []
            mv_sb = []
            for t in range(b_loc):
                tok = slice(t * L, (t + 1) * L)
                p = consts.tile([L, 1], i32, tag=f"pix{t}")
                nc.scalar.dma_start(out=p[:], in_=pix_h[tok, :])
                pix_sb.append(p)
                m = consts.tile([L, 2], f32, tag=f"mv{t}")
                nc.scalar.dma_start(out=m[:], in_=mv_h[tok, :])
                mv_sb.append(m)

            # out tiles; psg gathers are deferred into tile 1's taper stretch
            # as bus filler (see schedule below)
            outts = []
            for t in range(b_loc):
                outt = emb.tile([L, E], f32, tag="outt")
                outts.append(outt)

            # The psg gathers must execute LATE (they are the bus filler for
            # tile 1's taper stretch, where the pipeline otherwise idles).
            # A copy of the index tile that depends on a chosen stream chunk's
            # m_flat slot pins each gather after that chunk — the scheduler
            # cannot hoist it.
            pix2 = []
            for t in range(b_loc):
                p2 = consts.tile([L, 1], i32, tag=f"pix2_{t}")
                pix2.append(p2)

            def psg_dep(t, anchor_chunk):
                a = 8 * chunk_slots[anchor_chunk][-1]
                zm = small.tile([L, 1], f32, tag=f"zm{t}")
                nc.vector.tensor_scalar(
                    zm[:], m_flat[1][:, a:a + 1], 0.0, None, op0=Op.mult)
                zi = small.tile([L, 1], i32, tag=f"zi{t}")
                nc.vector.tensor_copy(out=zi[:], in_=zm[:])
                nc.vector.tensor_tensor(
                    out=pix2[t][:], in0=pix_sb[t][:], in1=zi[:], op=Op.add)

            def psg_gather(t):
                nc.gpsimd.indirect_dma_start(
                    out=outts[t][:], out_offset=None, in_=wz_h[:],
                    in_offset=IndirectOffsetOnAxis(ap=pix2[t][:, 0:1], axis=0),
                )

            def issue_chunk(t, c, lo, csz, pend):
                """Issue one chunk's load+accum+max+max_index; returns nothing.
                pend: list collecting deferred Pool-queue callbacks."""
                tok = slice(t * L, (t + 1) * L)
                lt = lpools[t].tile([L, bulk], f32, tag="lt")
                ldeng = nc.scalar if c % 2 else nc.sync
                ldeng.dma_start(out=lt[:, 0:csz], in_=logits_h[tok, lo:lo + csz])
                for sid in chunk_slots[c]:
                    lo_s, sz_s = slots[sid]
                    sl = slice(lo_s - lo, lo_s - lo + sz_s)
                    nc.gpsimd.dma_start(
                        out=lt[:, sl], in_=gumbel_h[tok, lo_s:lo_s + sz_s],
                        accum_op=Op.add)
                    s = slice(8 * sid, 8 * sid + 8)
                    nc.vector.max(out=m_flat[t][:, s], in_=lt[:, sl])
                    nc.vector.max_index(
                        out=i_flat[t][:, s], in_max=m_flat[t][:, s],
                        in_values=lt[:, sl])

            n_last = d.get("N_LAST", 2)  # stat slots folded serially in merge

            def tail_partial(t):
                """Winner among slots 0..n_slots-1-n_last: runs while the last
                (small) chunks' accumulates are still in flight."""
                Fp = 8 * (n_slots - n_last)
                # idx+base in f32 for the partial range (hidden behind the
                # last chunks' DMA flight)
                ibpa = small.tile([L, F], f32, tag="ibpa")
                nc.vector.tensor_copy(out=ibpa[:, 0:Fp], in_=i_flat[t][:, 0:Fp])
                nc.vector.tensor_tensor(
                    out=ibpa[:, 0:Fp], in0=ibpa[:, 0:Fp], in1=bases_f[:, 0:Fp],
                    op=Op.add)
                mm8a = small.tile([L, 8], f32, tag="mm8a")
                nc.vector.max(out=mm8a[:], in_=m_flat[t][:, 0:Fp])
                jj8a = small.tile([L, 8], u32, tag="jj8a")
                nc.vector.max_index(
                    out=jj8a[:], in_max=mm8a[:], in_values=m_flat[t][:, 0:Fp])
                jfa = small.tile([L, 1], f32, tag="jfa")
                nc.vector.tensor_copy(out=jfa[:], in_=jj8a[:, 0:1])
                sela = small.tile([L, F], f32, tag="sela")
                nc.vector.scalar_tensor_tensor(
                    out=sela[:, 0:Fp], in0=iota_f[:, 0:Fp], scalar=jfa[:, 0:1],
                    in1=ibpa[:, 0:Fp], op0=Op.is_equal, op1=Op.mult)
                hota = small.tile([L, 1], f32, tag="hota")
                nc.vector.reduce_max(out=hota[:], in_=sela[:, 0:Fp], axis=AX.X)
                return mm8a, hota

            def tail_merge(t, mm8a, hota):
                """Fold the last n_last chunks' max/argmax into the partial
                winner, one at a time in vocab order. Strict > keeps argmax
                first-occurrence tie semantics."""
                gcur, hcur = mm8a[:, 0:1], hota[:]
                for k in range(n_slots - n_last, n_slots):
                    sL = slice(8 * k, 8 * k + 1)
                    bet = small.tile([L, 1], f32, tag=f"bet{k}")
                    nc.vector.tensor_tensor(
                        out=bet[:], in0=m_flat[t][:, sL], in1=gcur, op=Op.is_gt)
                    ib1 = small.tile([L, 1], f32, tag=f"ib1_{k}")
                    nc.vector.tensor_scalar(
                        ib1[:], i_flat[t][:, sL], float(slots[k][0]), None,
                        op0=Op.add)
                    d1 = small.tile([L, 1], f32, tag=f"d1_{k}")
                    nc.vector.tensor_tensor(
                        out=d1[:], in0=ib1[:], in1=hcur, op=Op.subtract)
                    hnew = small.tile([L, 1], f32, tag=f"hnew{k}")
                    # hot = bet*(ibp_k - hot) + hot
                    nc.vector.scalar_tensor_tensor(
                        out=hnew[:], in0=bet[:], scalar=d1[:, 0:1], in1=hcur,
                        op0=Op.mult, op1=Op.add)
                    hcur = hnew[:]
                    if k < n_slots - 1:
                        gnew = small.tile([L, 1], f32, tag=f"gnew{k}")
                        nc.vector.tensor_tensor(
                            out=gnew[:], in0=m_flat[t][:, sL], in1=gcur, op=Op.max)
                        gcur = gnew[:]
                tokf = small.tile([L, 1], f32, tag="tokf")
                # tok_idx = hot*mask + (1-mask)*V
                nc.vector.tensor_scalar(
                    tokf[:], hcur, mv_sb[t][:, 0:1], None, op0=Op.mult)
                nc.vector.tensor_tensor(
                    out=tokf[:], in0=tokf[:], in1=mv_sb[t][:, 1:2], op=Op.add)
                toki = small.tile([L, 1], i32, tag="toki")
                nc.vector.tensor_copy(out=toki[:], in_=tokf[:])
                return toki

            def tail_gather(t, toki):
                nc.gpsimd.indirect_dma_start(
                    out=outts[t][:], out_offset=None, in_=w2z_h[:],
                    in_offset=IndirectOffsetOnAxis(ap=toki[:, 0:1], axis=0),
                    compute_op=Op.add,
                )

            def tail_store(t):
                tok = slice(t * L, (t + 1) * L)
                eng = {"pool": nc.gpsimd, "sp": nc.sync, "act": nc.scalar,
                       "dve": nc.vector}[store_eng]
                eng.dma_start(out=out_h[tok, :], in_=outts[t][:])

            m_flat = [None] * b_loc
            i_flat = [None] * b_loc
            for t in range(b_loc):
                mft = stats.tile([L, F], f32, tag="m_flat")
                ift = stats.tile([L, F], u32, tag="i_flat")
                m_flat[t], i_flat[t] = mft, ift

            # Global issue schedule: t0 bulk; then t0's taper interleaved 1:1
            # with t1's early bulk (so t0's latency-bound taper round trips
            # hide inside t1's streaming instead of blocking the in-order
            # queues); then the rest of t1. Tail compute is issued right after
            # the owning tile's chunks; t0's gather/store are placed a few
            # chunks into t1's remaining stream (their waits are met by then).
            n_taper = len(d.get("TAPER", TAPER))
            nb = nsp - n_taper
            part_slot = n_slots - 1 - n_last
            part_chunk = next(c for c in range(nsp)
                              if part_slot in chunk_slots[c])
            sched = [("chunk", 0, c) for c in range(nb)]
            for i in range(n_taper):
                if i < nsp:
                    sched.append(("chunk", 1, i))
                sched.append(("chunk", 0, nb + i))
                if nb + i == part_chunk:
                    sched.append(("partial", 0, 0))
                if nb + i == nsp - 1:
                    sched.append(("merge", 0, 0))
            a0 = d.get("ANCHOR0", 6)    # tile-1 chunk anchoring psg0's gather
            a1 = d.get("ANCHOR1", 13)   # tile-1 chunk anchoring psg1's gather
            for j, c in enumerate(range(n_taper, nsp)):
                if j == 0:
                    sched.append(("psgdep", 0, a0))
                    sched.append(("psg", 0, 0))
                    sched.append(("psgdep", 1, a1))
                    sched.append(("psg", 1, 0))
                if j == k_gather:
                    sched.append(("gather", 0, 0))
                if j == k_store:
                    sched.append(("store", 0, 0))
                sched.append(("chunk", 1, c))
                if c == part_chunk:
                    sched.append(("partial", 1, 0))
            sched += [("merge", 1, 0), ("gather", 1, 0), ("store", 1, 0)]

            part = [None] * b_loc
            toki = [None] * b_loc
            for kind, t, c in sched:
                if kind == "chunk":
                    lo, csz = spans[c]
                    issue_chunk(t, c, lo, csz, None)
                elif kind == "partial":
                    part[t] = tail_partial(t)
                elif kind == "merge":
                    toki[t] = tail_merge(t, *part[t])
                elif kind == "psgdep":
                    psg_dep(t, c)
                elif kind == "psg":
                    psg_gather(t)
                elif kind == "gather":
                    tail_gather(t, toki[t])
                elif kind == "store":
                    tail_store(t)

    return nc


_BUILD_CACHE = {}


def _get_module(dims_key=None, dims=None):
    key = dims_key
    if key not in _BUILD_CACHE:
        import concourse.bacc as bacc

        nc = bacc.Bacc("TRN2", target_bir_lowering=False, debug=False)
        _build(nc, dims)
        nc.compile()
        _BUILD_CACHE[key] = nc
    return _BUILD_CACHE[key]


_MAPS_CACHE = {}


def _nearest_maps():
    """Replicate the reference's f32 grid_sample-nearest index maps with jnp
    on the same backend the reference runs on (bit-exact by construction)."""
    if "maps" not in _MAPS_CACHE:
        import jax.numpy as jnp

        def nearest(size):
            lin = jnp.linspace(-1.0, 1.0, size)
            ix = ((lin + 1.0) * size - 1.0) / 2.0
            return np.asarray(jnp.clip(jnp.round(ix), 0, size - 1).astype(jnp.int32))

        _MAPS_CACHE["maps"] = (nearest(V), nearest(E))
    return _MAPS_CACHE["maps"]


def _aux_array(dims=None):
    _, slots, _ = _slots(dims or {})
    F = 8 * len(slots)
    iota = np.arange(F, dtype=np.float32)
    bases = np.repeat(np.array([lo for lo, _ in slots], dtype=np.float32), 8)
    row = np.concatenate([iota, bases])
    return np.ascontiguousarray(np.broadcast_to(row, (L, 2 * F)).astype(np.float32))


# test/dev hooks: set TRACE=True before calling kernel() to capture an NTFF
# profile; the BassKernelResults of the last run is stored in LAST_RESULT.
TRACE = False
LAST_RESULT = None


def kernel(logits, rwrt_attention_mask, psg_input_ids, word_embeddings, gumbel_noise):
    from concourse.bass_utils import run_bass_kernel_spmd

    logits = np.ascontiguousarray(np.asarray(logits, dtype=np.float32))
    gumbel = np.ascontiguousarray(np.asarray(gumbel_noise, dtype=np.float32))
    mask = np.asarray(rwrt_attention_mask, dtype=np.int32)
    psg = np.asarray(psg_input_ids, dtype=np.int32)
    wte = np.asarray(word_embeddings, dtype=np.float32)

    rowmap, colmap = _nearest_maps()
    zrow = np.zeros((1, E), dtype=np.float32)
    w2z = np.ascontiguousarray(np.vstack([wte[rowmap][:, colmap], zrow]))
    wz = np.ascontiguousarray(np.vstack([wte, zrow]))

    # passage branch index arithmetic (exact integer ops, O(B*L))
    psg_roll = np.roll(psg, 1, axis=1)
    psg_roll[:, 0] = 1
    flipped = 1 - mask[:, ::-1]
    extr = flipped * psg_roll
    shifts = mask.sum(axis=1)
    pos = (np.arange(L)[None, :] - shifts[:, None]) % L
    trunc = np.take_along_axis(extr, pos, axis=1)
    flag = np.cumsum(trunc != 0, axis=1) > 0
    pix = np.where(flag, trunc, V).astype(np.int32)

    maskf = mask.astype(np.float32)
    vinv = (1.0 - maskf) * float(V)
    mv = np.ascontiguousarray(
        np.stack([maskf, vinv], axis=-1).astype(np.float32))  # [B, L, 2]
    aux = _aux_array()

    nc = _get_module()

    in_maps = []
    for m in range(N_CORES):
        sl = slice(m * B_LOC, (m + 1) * B_LOC)
        in_maps.append({
            "logits": logits[sl].reshape(B_LOC * L, V),
            "gumbel": gumbel[sl].reshape(B_LOC * L, V),
            "pix": np.ascontiguousarray(pix[sl].reshape(B_LOC * L, 1)),
            "mv": np.ascontiguousarray(mv[sl].reshape(B_LOC * L, 2)),
            "aux": aux,
            "w2z": w2z,
            "wz": wz,
        })

    global LAST_RESULT
    try:
        LAST_RESULT = run_bass_kernel_spmd(nc, in_maps, list(range(N_CORES)), trace=TRACE)
    except Exception:
        # the axon-relayed device occasionally reports a transient
        # NRT_EXEC_UNIT_UNRECOVERABLE on the first execution after long
        # sessions; a straight re-run recovers it
        import time as _time

        _time.sleep(2.0)
        LAST_RESULT = run_bass_kernel_spmd(nc, in_maps, list(range(N_CORES)), trace=TRACE)
    res = LAST_RESULT.results
    out = np.concatenate(
        [res[m]["out"].reshape(B_LOC, L, E) for m in range(N_CORES)], axis=0
    )
    return out


# revision 49
# speedup vs baseline: 1.0454x; 1.0041x over previous
"""Trainium2 Bass kernel: gumbel-softmax-argmax embedding lookup (end-to-end).

Reference math (nn_End2End_49495203119139):
    hot  = argmax_V(softmax((logits + gumbel)/tau))       == argmax_V(logits+gumbel)
    row  = grid_sample-nearest index map of hot            == ROWMAP[hot]  (LUT)
    tok_emb = W[row][:, col_map]
    inputs_embeds = tok_emb * mask
    psg_roll = roll(psg_ids, 1, axis=1); psg_roll[:,0] = 1
    extr  = (1 - mask[:, ::-1]) * psg_roll
    trunc = rotate_right(extr, shifts) with shifts = mask.sum(-1)   (per row)
    flag  = cumsum(trunc != 0, -1) > 0
    out   = inputs_embeds + where(flag, W[trunc], 0)

Sharding: data-parallel over batch. B=16 over 8 cores -> 2 batch rows
(= 2 token tiles of 128) per core; embedding tables replicated.

Host precomputes (cheap, O(B*L) index arithmetic + one-time table reshuffles):
  - W2Z [V+1,E] = W[ROWMAP][:, COLMAP] with a zero row appended at index V
  - WZ  [V+1,E] = W with a zero row appended at index V
  - psg_idx [B,L] = flag ? trunc : V     (zero-row redirect replaces `where`)
  - mask_f  [B,L], vinv = (1-mask)*V     (token index redirect coefficients)

Per-core device plan (memory regime: streaming logits+gumbel, 66 MB/core,
~183 us HBM floor at 360 GB/s per core; sim/HW 204.0 us vs 212.4 baseline):
  - stream the vocab in chunks per 128-token tile (bulk 2048 wide, with a
    gradually tapered chunk schedule at each tile's end): HWDGE-load the
    logits chunk (ACT/SP queues alternate), add the gumbel chunk in the DMA
    datapath (SWDGE CCE inline add, <=2048 elements per descriptor), then
    DVE Max + MaxIndex written in place into flat [128, 8*nchunks] stats
    tiles. The taper keeps the DVE pipeline (which runs ~1 chunk + 900 ns
    sem behind the bus) nearly drained when the final accumulate lands.
  - tile 0's taper is interleaved 1:1 with tile 1's early bulk chunks so
    its latency-bound round trips hide inside tile 1's streaming; separate
    per-tile load-buffer pools keep tile 1's loads independent of tile 0's
    DVE progress.
  - tail per tile, split in two: the winner over all but the last 3 chunks
    (Max+MaxIndex over the flat stats + iota-select of idx+base) runs while
    those chunks' accumulates are still in flight; the merge then folds the
    last 3 chunks with strict-> compares (argmax first-occurrence ties).
    tok_idx = hot*mask + (1-mask)*V; one indirect gather of W2Z[tok_idx]
    with compute_op=add accumulates the token embedding onto the
    psg embedding (gathered from WZ[psg_idx]) in SBUF; Pool-queue store.
  - the psg gathers are pinned LATE via an artificial data dependency on a
    mid-stream chunk's stats (the scheduler would otherwise hoist them to
    t=0); they are the bus filler for tile 1's taper stretch, where the
    load->accum->max round trips otherwise leave the bus idle.
"""

import numpy as np

B = 16
L = 128
V = 32128
E = 768
N_CORES = 8
B_LOC = B // N_CORES          # batch rows (= 128-token tiles) per core
BULK = 2048                   # bulk vocab chunk width (<=2048: one CCE-add
                              # descriptor per partition row)
# long gradual taper: the DVE pipeline runs ~1 chunk + 900ns behind the DMA
# stream, so chunk sizes must shrink smoothly toward the end for the final
# max/max_index work to be tiny when the last accumulate lands
TAPER = (2048, 1536, 1280, 1024, 768, 512, 256, 128)
NEG = -3.0e38


def _spans(d):
    bulk = d.get("BULK", BULK)
    taper = list(d.get("TAPER", TAPER))
    t_sum = sum(taper)
    assert (V - t_sum) % bulk == 0, (V, t_sum, bulk)
    spans = [(c * bulk, bulk) for c in range((V - t_sum) // bulk)]
    lo = V - t_sum
    for s in taper:
        spans.append((lo, s))
        lo += s
    assert lo == V
    return spans


def _slots(d):
    """Stats slots: each load span splits into <=SPLIT-wide sub-slots so the
    DVE max/max_index pipeline quantum stays small. Returns (spans, slots,
    chunk_slots) where chunk_slots[c] lists slot indices of load chunk c."""
    spans = _spans(d)
    split = d.get("SPLIT", 0)
    slots = []
    chunk_slots = []
    for lo, csz in spans:
        ids = []
        off = 0
        while off < csz:
            sz = min(split, csz - off) if split else csz
            ids.append(len(slots))
            slots.append((lo + off, sz))
            off += sz
        chunk_slots.append(ids)
    return spans, slots, chunk_slots


def _build(nc_mod, dims=None):
    import concourse.tile as tile
    from concourse import mybir
    from concourse.bass import IndirectOffsetOnAxis

    d = dims or {}
    spans, slots, chunk_slots = _slots(d)
    nsp = len(spans)
    n_slots = len(slots)
    F = 8 * n_slots
    b_loc = d.get("B_LOC", B_LOC)
    rows = b_loc * L
    lbufs = d.get("LBUFS", 6)
    bulk = d.get("BULK", BULK)
    # Pool-queue positions (within the NEXT tile's accumulate stream) at which
    # the previous tile's tail gather / store are interleaved
    k_gather = d.get("K_GATHER", 2)
    k_store = d.get("K_STORE", 4)
    store_eng = d.get("STORE_ENG", "pool")

    nc = nc_mod
    f32 = mybir.dt.float32
    i32 = mybir.dt.int32
    u32 = mybir.dt.uint32
    Op = mybir.AluOpType
    AX = mybir.AxisListType

    logits_h = nc.dram_tensor("logits", [rows, V], f32, kind="ExternalInput")
    gumbel_h = nc.dram_tensor("gumbel", [rows, V], f32, kind="ExternalInput")
    pix_h = nc.dram_tensor("pix", [rows, 1], i32, kind="ExternalInput")
    mv_h = nc.dram_tensor("mv", [rows, 2], f32, kind="ExternalInput")
    aux_h = nc.dram_tensor("aux", [L, 2 * F], f32, kind="ExternalInput")
    w2z_h = nc.dram_tensor("w2z", [V + 1, E], f32, kind="ExternalInput")
    wz_h = nc.dram_tensor("wz", [V + 1, E], f32, kind="ExternalInput")
    out_h = nc.dram_tensor("out", [rows, E], f32, kind="ExternalOutput")

    with tile.TileContext(nc) as tc:
        with (
            tc.tile_pool(name="lpool0", bufs=lbufs) as lpool0,
            tc.tile_pool(name="lpool1", bufs=lbufs) as lpool1,
            tc.tile_pool(name="stats", bufs=2) as stats,
            tc.tile_pool(name="small", bufs=2) as small,
            tc.tile_pool(name="emb", bufs=2) as emb,
            tc.tile_pool(name="consts", bufs=1) as consts,
        ):
            lpools = [lpool0, lpool1]
            # ---- tiny constant loads (ACT queue, ahead of its odd-chunk
            # loads; they land on the bus before the first big transfer) ----
            aux_sb = consts.tile([L, 2 * F], f32)
            nc.scalar.dma_start(out=aux_sb[:], in_=aux_h[:])
            iota_f = aux_sb[:, 0:F]
            bases_f = aux_sb[:, F:2 * F]
            pix_sb = []
            mv_sb = []
            for t in range(b_loc):
                tok = slice(t * L, (t + 1) * L)
                p = consts.tile([L, 1], i32, tag=f"pix{t}")
                nc.scalar.dma_start(out=p[:], in_=pix_h[tok, :])
                pix_sb.append(p)
                m = consts.tile([L, 2], f32, tag=f"mv{t}")
                nc.scalar.dma_start(out=m[:], in_=mv_h[tok, :])
                mv_sb.append(m)

            # out tiles; psg gathers are deferred into tile 1's taper stretch
            # as bus filler (see schedule below)
            outts = []
            for t in range(b_loc):
                outt = emb.tile([L, E], f32, tag="outt")
                outts.append(outt)

            # The psg gathers must execute LATE (they are the bus filler for
            # tile 1's taper stretch, where the pipeline otherwise idles).
            # A copy of the index tile that depends on a chosen stream chunk's
            # m_flat slot pins each gather after that chunk — the scheduler
            # cannot hoist it.
            pix2 = []
            for t in range(b_loc):
                p2 = consts.tile([L, 1], i32, tag=f"pix2_{t}")
                pix2.append(p2)

            def psg_dep(t, anchor_chunk):
                a = 8 * chunk_slots[anchor_chunk][-1]
                zm = small.tile([L, 1], f32, tag=f"zm{t}")
                nc.vector.tensor_scalar(
                    zm[:], m_flat[1][:, a:a + 1], 0.0, None, op0=Op.mult)
                zi = small.tile([L, 1], i32, tag=f"zi{t}")
                nc.vector.tensor_copy(out=zi[:], in_=zm[:])
                nc.vector.tensor_tensor(
                    out=pix2[t][:], in0=pix_sb[t][:], in1=zi[:], op=Op.add)

            def psg_gather(t):
                nc.gpsimd.indirect_dma_start(
                    out=outts[t][:], out_offset=None, in_=wz_h[:],
                    in_offset=IndirectOffsetOnAxis(ap=pix2[t][:, 0:1], axis=0),
                )

            def issue_chunk(t, c, lo, csz, pend):
                """Issue one chunk's load+accum+max+max_index; returns nothing.
                pend: list collecting deferred Pool-queue callbacks."""
                tok = slice(t * L, (t + 1) * L)
                lt = lpools[t].tile([L, bulk], f32, tag="lt")
                ldeng = nc.scalar if c % 2 else nc.sync
                ldeng.dma_start(out=lt[:, 0:csz], in_=logits_h[tok, lo:lo + csz])
                for sid in chunk_slots[c]:
                    lo_s, sz_s = slots[sid]
                    sl = slice(lo_s - lo, lo_s - lo + sz_s)
                    nc.gpsimd.dma_start(
                        out=lt[:, sl], in_=gumbel_h[tok, lo_s:lo_s + sz_s],
                        accum_op=Op.add)
                    s = slice(8 * sid, 8 * sid + 8)
                    nc.vector.max(out=m_flat[t][:, s], in_=lt[:, sl])
                    nc.vector.max_index(
                        out=i_flat[t][:, s], in_max=m_flat[t][:, s],
                        in_values=lt[:, sl])

            n_last = d.get("N_LAST", 3)  # stat slots folded serially in merge

            def tail_partial(t):
                """Winner among slots 0..n_slots-1-n_last: runs while the last
                (small) chunks' accumulates are still in flight."""
                Fp = 8 * (n_slots - n_last)
                # idx+base in f32 for the partial range (hidden behind the
                # last chunks' DMA flight)
                ibpa = small.tile([L, F], f32, tag="ibpa")
                nc.vector.tensor_copy(out=ibpa[:, 0:Fp], in_=i_flat[t][:, 0:Fp])
                nc.vector.tensor_tensor(
                    out=ibpa[:, 0:Fp], in0=ibpa[:, 0:Fp], in1=bases_f[:, 0:Fp],
                    op=Op.add)
                mm8a = small.tile([L, 8], f32, tag="mm8a")
                nc.vector.max(out=mm8a[:], in_=m_flat[t][:, 0:Fp])
                jj8a = small.tile([L, 8], u32, tag="jj8a")
                nc.vector.max_index(
                    out=jj8a[:], in_max=mm8a[:], in_values=m_flat[t][:, 0:Fp])
                jfa = small.tile([L, 1], f32, tag="jfa")
                nc.vector.tensor_copy(out=jfa[:], in_=jj8a[:, 0:1])
                sela = small.tile([L, F], f32, tag="sela")
                nc.vector.scalar_tensor_tensor(
                    out=sela[:, 0:Fp], in0=iota_f[:, 0:Fp], scalar=jfa[:, 0:1],
                    in1=ibpa[:, 0:Fp], op0=Op.is_equal, op1=Op.mult)
                hota = small.tile([L, 1], f32, tag="hota")
                nc.vector.reduce_max(out=hota[:], in_=sela[:, 0:Fp], axis=AX.X)
                return mm8a, hota

            def tail_merge(t, mm8a, hota):
                """Fold the last n_last chunks' max/argmax into the partial
                winner, one at a time in vocab order. Strict > keeps argmax
                first-occurrence tie semantics."""
                gcur, hcur = mm8a[:, 0:1], hota[:]
                for k in range(n_slots - n_last, n_slots):
                    sL = slice(8 * k, 8 * k + 1)
                    bet = small.tile([L, 1], f32, tag=f"bet{k}")
                    nc.vector.tensor_tensor(
                        out=bet[:], in0=m_flat[t][:, sL], in1=gcur, op=Op.is_gt)
                    ib1 = small.tile([L, 1], f32, tag=f"ib1_{k}")
                    nc.vector.tensor_scalar(
                        ib1[:], i_flat[t][:, sL], float(slots[k][0]), None,
                        op0=Op.add)
                    d1 = small.tile([L, 1], f32, tag=f"d1_{k}")
                    nc.vector.tensor_tensor(
                        out=d1[:], in0=ib1[:], in1=hcur, op=Op.subtract)
                    hnew = small.tile([L, 1], f32, tag=f"hnew{k}")
                    # hot = bet*(ibp_k - hot) + hot
                    nc.vector.scalar_tensor_tensor(
                        out=hnew[:], in0=bet[:], scalar=d1[:, 0:1], in1=hcur,
                        op0=Op.mult, op1=Op.add)
                    hcur = hnew[:]
                    if k < n_slots - 1:
                        gnew = small.tile([L, 1], f32, tag=f"gnew{k}")
                        nc.vector.tensor_tensor(
                            out=gnew[:], in0=m_flat[t][:, sL], in1=gcur, op=Op.max)
                        gcur = gnew[:]
                tokf = small.tile([L, 1], f32, tag="tokf")
                # tok_idx = hot*mask + (1-mask)*V
                nc.vector.tensor_scalar(
                    tokf[:], hcur, mv_sb[t][:, 0:1], None, op0=Op.mult)
                nc.vector.tensor_tensor(
                    out=tokf[:], in0=tokf[:], in1=mv_sb[t][:, 1:2], op=Op.add)
                toki = small.tile([L, 1], i32, tag="toki")
                nc.vector.tensor_copy(out=toki[:], in_=tokf[:])
                return toki

            def tail_gather(t, toki):
                nc.gpsimd.indirect_dma_start(
                    out=outts[t][:], out_offset=None, in_=w2z_h[:],
                    in_offset=IndirectOffsetOnAxis(ap=toki[:, 0:1], axis=0),
                    compute_op=Op.add,
                )

            def tail_store(t):
                tok = slice(t * L, (t + 1) * L)
                eng = {"pool": nc.gpsimd, "sp": nc.sync, "act": nc.scalar,
                       "dve": nc.vector}[store_eng]
                eng.dma_start(out=out_h[tok, :], in_=outts[t][:])

            m_flat = [None] * b_loc
            i_flat = [None] * b_loc
            for t in range(b_loc):
                mft = stats.tile([L, F], f32, tag="m_flat")
                ift = stats.tile([L, F], u32, tag="i_flat")
                m_flat[t], i_flat[t] = mft, ift

            # Global issue schedule: t0 bulk; then t0's taper interleaved 1:1
            # with t1's early bulk (so t0's latency-bound taper round trips
            # hide inside t1's streaming instead of blocking the in-order
            # queues); then the rest of t1. Tail compute is issued right after
            # the owning tile's chunks; t0's gather/store are placed a few
            # chunks into t1's remaining stream (their waits are met by then).
            n_taper = len(d.get("TAPER", TAPER))
            nb = nsp - n_taper
            part_slot = n_slots - 1 - n_last
            part_chunk = next(c for c in range(nsp)
                              if part_slot in chunk_slots[c])
            mode = d.get("SCHED", "old")
            if mode == "dualtaper":
                # Both tiles' tapers interleaved 1:1, tile 0 offset 'lead'
                # chunks earlier so its tail (merge/gather/store) fills the
                # end-region bus bubbles without contending with tile 1's.
                lead = d.get("LEAD", 2)
                K = nsp - n_taper - lead
                order = [(0, c) for c in range(nb)]
                order += [(1, c) for c in range(K)]
                for i in range(n_taper):
                    order.append((0, nb + i))
                    order.append((1, K + i))
                order += [(1, c) for c in range(K + n_taper, nsp)]
                a0 = d.get("ANCHOR0", max(0, K - 4))
                a1 = d.get("ANCHOR1", min(nsp - 3, K + 3))
                sched = []
                for t, c in order:
                    sched.append(("chunk", t, c))
                    if c == part_chunk:
                        sched.append(("partial", t, 0))
                    if t == 0 and c == nsp - 1:
                        sched += [("merge", 0, 0), ("gather", 0, 0),
                                  ("store", 0, 0)]
                    if t == 1 and c == 0:
                        sched += [("psgdep", 0, a0), ("psg", 0, 0),
                                  ("psgdep", 1, a1), ("psg", 1, 0)]
                sched += [("merge", 1, 0), ("gather", 1, 0), ("store", 1, 0)]
            else:
                sched = [("chunk", 0, c) for c in range(nb)]
                for i in range(n_taper):
                    if i < nsp:
                        sched.append(("chunk", 1, i))
                    sched.append(("chunk", 0, nb + i))
                    if nb + i == part_chunk:
                        sched.append(("partial", 0, 0))
                    if nb + i == nsp - 1:
                        sched.append(("merge", 0, 0))
                a0 = d.get("ANCHOR0", 6)
                a1 = d.get("ANCHOR1", 13)
                for j, c in enumerate(range(n_taper, nsp)):
                    if j == 0:
                        sched.append(("psgdep", 0, a0))
                        sched.append(("psg", 0, 0))
                        sched.append(("psgdep", 1, a1))
                        sched.append(("psg", 1, 0))
                    if j == k_gather:
                        sched.append(("gather", 0, 0))
                    if j == k_store:
                        sched.append(("store", 0, 0))
                    sched.append(("chunk", 1, c))
                    if c == part_chunk:
                        sched.append(("partial", 1, 0))
                sched += [("merge", 1, 0), ("gather", 1, 0), ("store", 1, 0)]

            part = [None] * b_loc
            toki = [None] * b_loc
            for kind, t, c in sched:
                if kind == "chunk":
                    lo, csz = spans[c]
                    issue_chunk(t, c, lo, csz, None)
                elif kind == "partial":
                    part[t] = tail_partial(t)
                elif kind == "merge":
                    toki[t] = tail_merge(t, *part[t])
                elif kind == "psgdep":
                    psg_dep(t, c)
                elif kind == "psg":
                    psg_gather(t)
                elif kind == "gather":
                    tail_gather(t, toki[t])
                elif kind == "store":
                    tail_store(t)

    return nc


_BUILD_CACHE = {}


def _get_module(dims_key=None, dims=None):
    key = dims_key
    if key not in _BUILD_CACHE:
        import concourse.bacc as bacc

        nc = bacc.Bacc("TRN2", target_bir_lowering=False, debug=False)
        _build(nc, dims)
        nc.compile()
        _BUILD_CACHE[key] = nc
    return _BUILD_CACHE[key]


_MAPS_CACHE = {}


def _nearest_maps():
    """Replicate the reference's f32 grid_sample-nearest index maps with jnp
    on the same backend the reference runs on (bit-exact by construction)."""
    if "maps" not in _MAPS_CACHE:
        import jax.numpy as jnp

        def nearest(size):
            lin = jnp.linspace(-1.0, 1.0, size)
            ix = ((lin + 1.0) * size - 1.0) / 2.0
            return np.asarray(jnp.clip(jnp.round(ix), 0, size - 1).astype(jnp.int32))

        _MAPS_CACHE["maps"] = (nearest(V), nearest(E))
    return _MAPS_CACHE["maps"]


def _aux_array(dims=None):
    _, slots, _ = _slots(dims or {})
    F = 8 * len(slots)
    iota = np.arange(F, dtype=np.float32)
    bases = np.repeat(np.array([lo for lo, _ in slots], dtype=np.float32), 8)
    row = np.concatenate([iota, bases])
    return np.ascontiguousarray(np.broadcast_to(row, (L, 2 * F)).astype(np.float32))


# test/dev hooks: set TRACE=True before calling kernel() to capture an NTFF
# profile; the BassKernelResults of the last run is stored in LAST_RESULT.
TRACE = False
LAST_RESULT = None


def kernel(logits, rwrt_attention_mask, psg_input_ids, word_embeddings, gumbel_noise):
    from concourse.bass_utils import run_bass_kernel_spmd

    logits = np.ascontiguousarray(np.asarray(logits, dtype=np.float32))
    gumbel = np.ascontiguousarray(np.asarray(gumbel_noise, dtype=np.float32))
    mask = np.asarray(rwrt_attention_mask, dtype=np.int32)
    psg = np.asarray(psg_input_ids, dtype=np.int32)
    wte = np.asarray(word_embeddings, dtype=np.float32)

    rowmap, colmap = _nearest_maps()
    zrow = np.zeros((1, E), dtype=np.float32)
    w2z = np.ascontiguousarray(np.vstack([wte[rowmap][:, colmap], zrow]))
    wz = np.ascontiguousarray(np.vstack([wte, zrow]))

    # passage branch index arithmetic (exact integer ops, O(B*L))
    psg_roll = np.roll(psg, 1, axis=1)
    psg_roll[:, 0] = 1
    flipped = 1 - mask[:, ::-1]
    extr = flipped * psg_roll
    shifts = mask.sum(axis=1)
    pos = (np.arange(L)[None, :] - shifts[:, None]) % L
    trunc = np.take_along_axis(extr, pos, axis=1)
    flag = np.cumsum(trunc != 0, axis=1) > 0
    pix = np.where(flag, trunc, V).astype(np.int32)

    maskf = mask.astype(np.float32)
    vinv = (1.0 - maskf) * float(V)
    mv = np.ascontiguousarray(
        np.stack([maskf, vinv], axis=-1).astype(np.float32))  # [B, L, 2]
    aux = _aux_array()

    nc = _get_module()

    in_maps = []
    for m in range(N_CORES):
        sl = slice(m * B_LOC, (m + 1) * B_LOC)
        in_maps.append({
            "logits": logits[sl].reshape(B_LOC * L, V),
            "gumbel": gumbel[sl].reshape(B_LOC * L, V),
            "pix": np.ascontiguousarray(pix[sl].reshape(B_LOC * L, 1)),
            "mv": np.ascontiguousarray(mv[sl].reshape(B_LOC * L, 2)),
            "aux": aux,
            "w2z": w2z,
            "wz": wz,
        })

    global LAST_RESULT
    try:
        LAST_RESULT = run_bass_kernel_spmd(nc, in_maps, list(range(N_CORES)), trace=TRACE)
    except Exception:
        # the axon-relayed device occasionally reports a transient
        # NRT_EXEC_UNIT_UNRECOVERABLE on the first execution after long
        # sessions; a straight re-run recovers it
        import time as _time

        _time.sleep(2.0)
        LAST_RESULT = run_bass_kernel_spmd(nc, in_maps, list(range(N_CORES)), trace=TRACE)
    res = LAST_RESULT.results
    out = np.concatenate(
        [res[m]["out"].reshape(B_LOC, L, E) for m in range(N_CORES)], axis=0
    )
    return out
